# revision 27
# baseline (speedup 1.0000x reference)
"""MoE block (B=16, C=192, H=W=32, E=8, top-2, 3x3 same-conv experts) on 8 trn2 cores.

Strategy (v7, 1-D Winograd F(2,3) over image columns):
  - Router + top-2 combine on host; conv linearity folds the expert mix
    into ONE conv per sample (combined weights Wc). 2 convs/core.
  - Each conv is computed with 1-D Winograd F(2,3) along columns:
    output column pairs (2t, 2t+1) come from 4 products m_0..m_3 with
      V_0 = xe[t]-xe[t+1], V_1 = xo[t]+xe[t+1],
      V_2 = xe[t+1]-xo[t], V_3 = xo[t]-xo[t+1]
    (xe/xo = even/odd columns of the 34-wide zero-padded row), and
      y0 = m0+m1+m2,  y1 = m1-m2-m3.
    MACs drop 1.5x vs direct: 160 matmuls x 192 rows = 30720 streamed
    rows (vs 43008 for the direct form).
  - m_j[pos, och] = sum_{ich, dy} V_j[ich, pos+16*dy] * U_j[dy, ich, och]
    with U_j = G-transformed combined weights (host-computed, bf16).
    Contraction (192 ch x 3 dy = 576) in 5 K-chunks per j: 3 full chunks
    for ch 0-127 (dy via flat +16 shifts of the V image), ch 128-191
    packed (dy0,dy1) in one chunk using a dup tile whose upper 64
    partitions hold V pre-shifted by one V-row, plus one K=64 chunk (dy2).
  - V images are computed ON DEVICE by DVE tensor ops from de-interleaved
    xe/xo tiles (all APs contiguous bf16 -> 2x DVE mode). Host pre-shifts
    the dup-source tiles (xbd) so one DVE op fills both dup halves.
  - Sample-0 j0 V tiles come from host (head1/head2 DMAs) so the first
    matmul is gated only by one small DMA chain.
  - y transform: t01=m0+m1 / t2=m1-m2 on Pool, y0=t01+m2 / y1=t2-m3 on
    DVE, per (sample, block-pair) on [128,384] PSUM tiles; outputs bf16.
  - PE p-state: warmup matmuls (Pool memset + 14x192) anchor the ramp so
    all real matmuls run at 2.4 GHz.
"""

import numpy as np

B, C, H, W = 16, 192, 32, 32
E, TOPK = 8, 2
NCORES = 8
S = B // NCORES          # samples per core
NB = 4                   # position blocks of 128 (512 positions = 32 rows x 16 col-pairs)
NP = 2                   # block pairs
VSZ = 544                # V image: 34 rows x 16 col-pairs
XSZ = 578                # xe/xo: 34 rows x 17
P1 = 18 * 17             # split piece 1: xe/xo rows 0-17 (306 elems each)
P2 = 16 * 17             # split piece 2: xe/xo rows 18-33 (272 elems each)
XW = 2 * (P1 + P2)       # 1156; s0 layout [xe-p1|xo-p1|xe-p2|xo-p2], s1 [xe|xo]
UW = 5 * 192             # u columns per j

WARMUP_NS = [192] * 14
CFG = {
    "sp_order": "u_first", "s0_struct": "pair", "vd1_pos": "mid",
    "dma_order": [
        ("s", "h1"), ("s", "h2"), ("p", "xab0p1"), ("s", "uj1a"),
        ("p", "xbd0p1"), ("s", "uj1b"), ("p", "xab0p2"), ("p", "xbd0p2"),
        ("s", "uj2"), ("s", "uj3"), ("s", "xab1"), ("s", "xbd1"),
        ("s", "u1j01"), ("s", "u1j1"), ("s", "u1j2"), ("s", "u1j3"),
    ],
}

_cache = {}


def _build_module():
    import concourse.tile as tile
    from concourse import bacc, mybir

    f32 = mybir.dt.float32
    bf16 = mybir.dt.bfloat16

    nc = bacc.Bacc("TRN2", target_bir_lowering=False, debug=False, num_devices=NCORES)
    xab_d = nc.dram_tensor("xab", [S, 128, XW], bf16, kind="ExternalInput")
    xbd_d = nc.dram_tensor("xbd", [S, 128, XW], bf16, kind="ExternalInput")
    u_d = nc.dram_tensor("u", [S, 128, 4 * UW], bf16, kind="ExternalInput")
    h1_d = nc.dram_tensor("h1", [1, 128, VSZ + 3 * 192], bf16, kind="ExternalInput")
    h2_d = nc.dram_tensor("h2", [1, 128, VSZ + 2 * 192], bf16, kind="ExternalInput")
    out_d = nc.dram_tensor("out", [S, NP, 128, 768], bf16, kind="ExternalOutput")

    with tile.TileContext(nc) as tc:
        with (
            tc.tile_pool(name="img", bufs=1) as img,
            tc.tile_pool(name="vt", bufs=1) as vt,
            tc.tile_pool(name="win", bufs=1) as win,
            tc.tile_pool(name="tt", bufs=1) as tt,
            tc.tile_pool(name="st", bufs=1) as st,
            tc.tile_pool(name="cst", bufs=1) as cst,
            tc.tile_pool(name="ps", bufs=8, space="PSUM") as ps,
        ):
            XAB, XBD, Wt, V, D = {}, {}, {}, {}, {}
            for s in range(S):
                XAB[s] = img.tile([128, XW], bf16, name=f"xab_{s}", tag=f"xab_{s}")
                XBD[s] = img.tile([128, XW], bf16, name=f"xbd_{s}", tag=f"xbd_{s}")
                Wt[s] = win.tile([128, 4 * UW], bf16, name=f"u_{s}", tag=f"u_{s}")
                for j in range(4):
                    V[(s, j)] = vt.tile([128, VSZ], bf16, name=f"v_{s}_{j}",
                                        tag=f"v_{s}_{j}")
                    D[(s, j)] = vt.tile([128, VSZ], bf16, name=f"d_{s}_{j}",
                                        tag=f"d_{s}_{j}")
            H1 = win.tile([128, VSZ + 3 * 192], bf16, name="h1", tag="h1")
            H2 = win.tile([128, VSZ + 2 * 192], bf16, name="h2", tag="h2")
            TT, MC, ST = {}, {}, {}
            for s in range(S):
                for p in range(NP):
                    TT[(s, p)] = tt.tile([128, 768], f32, name=f"tt_{s}_{p}",
                                         tag=f"tt_{s}_{p}")
                    MC[(s, p)] = tt.tile([128, 1152], f32, name=f"mc_{s}_{p}",
                                         tag=f"mc_{s}_{p}")
                    ST[(s, p)] = st.tile([128, 768], bf16, name=f"st_{s}_{p}",
                                         tag=f"st_{s}_{p}")

            # --- input DMAs ------------------------------------------------
            # SP/HWDGE: head tiles (gate the first matmuls), then u chunks in
            # consumption order. Pool/SWDGE: the xe/xo image tiles.
            # --- PE warmup (anchor the p-state ramp early; Pool memset is
            # cheap and must precede Pool's SWDGE descriptor generation) ----
            scr = cst.tile([128, 192], bf16, name="scr", tag="scr")
            nc.gpsimd.memset(scr[:], 0.0)

            # Input DMAs: one SP/HWDGE FIFO in exact consumption order (the
            # serialized DMA-engine model then honours it). Pool's SWDGE path
            # is slower to first-byte (preamble memsets + 1038ns descgen).
            C3 = 3 * 192  # start of c3 within a j-group of u columns
            SPLIT = 2 * P1  # s0 piece boundary within the 1224-col layout
            sp_pieces = {
                "h1": lambda: nc.sync.dma_start(H1[:], h1_d[0]),
                "h2": lambda: nc.sync.dma_start(H2[:], h2_d[0]),
                "uj1a": lambda: nc.sync.dma_start(
                    Wt[0][:, UW : UW + C3], u_d[0, :, UW : UW + C3]),
                "uj1b": lambda: nc.sync.dma_start(
                    Wt[0][:, UW + C3 : 2 * UW], u_d[0, :, UW + C3 : 2 * UW]),
                "uj2": lambda: nc.sync.dma_start(
                    Wt[0][:, 2 * UW : 3 * UW], u_d[0, :, 2 * UW : 3 * UW]),
                "uj3": lambda: nc.sync.dma_start(
                    Wt[0][:, 3 * UW : 4 * UW], u_d[0, :, 3 * UW : 4 * UW]),
                "uj3a": lambda: nc.sync.dma_start(
                    Wt[0][:, 3 * UW : 3 * UW + C3], u_d[0, :, 3 * UW : 3 * UW + C3]),
                "uj3b": lambda: nc.sync.dma_start(
                    Wt[0][:, 3 * UW + C3 : 4 * UW], u_d[0, :, 3 * UW + C3 : 4 * UW]),
                "xab1": lambda: nc.sync.dma_start(XAB[1][:], xab_d[1]),
                "u1j01": lambda: nc.sync.dma_start(
                    Wt[1][:, 0 : UW], u_d[1, :, 0 : UW]),
                "xbd1": lambda: nc.sync.dma_start(XBD[1][:], xbd_d[1]),
                "u1j1": lambda: nc.sync.dma_start(
                    Wt[1][:, UW : 2 * UW], u_d[1, :, UW : 2 * UW]),
                "u1j2": lambda: nc.sync.dma_start(
                    Wt[1][:, 2 * UW : 3 * UW], u_d[1, :, 2 * UW : 3 * UW]),
                "u1j3": lambda: nc.sync.dma_start(
                    Wt[1][:, 3 * UW : 4 * UW], u_d[1, :, 3 * UW : 4 * UW]),
            }
            pool_pieces = {
                "xab0p1": lambda: nc.gpsimd.dma_start(
                    XAB[0][:, 0:SPLIT], xab_d[0, :, 0:SPLIT]),
                "xbd0p1": lambda: nc.gpsimd.dma_start(
                    XBD[0][:, 0:SPLIT], xbd_d[0, :, 0:SPLIT]),
                "xab0p2": lambda: nc.gpsimd.dma_start(
                    XAB[0][:, SPLIT:XW], xab_d[0, :, SPLIT:XW]),
                "xbd0p2": lambda: nc.gpsimd.dma_start(
                    XBD[0][:, SPLIT:XW], xbd_d[0, :, SPLIT:XW]),
            }
            # interleaved emission honoring per-queue order from CFG
            for q, nm in CFG["dma_order"]:
                (sp_pieces if q == "s" else pool_pieces)[nm]()

            # --- PSUM m tiles (8 banks, cycled; alloc order = write order) --
            m = {}
            for s in range(S):
                for j in range(4):
                    for p in range(NP):
                        m[(s, p, j)] = ps.tile([128, 384], f32,
                                               name=f"m_{s}_{p}_{j}", tag="m")

            for n in WARMUP_NS:
                nc.tensor.matmul(m[(0, 0, 0)][:, 0:n], scr[:, 0:128], scr[:, 0:n],
                                 start=True, stop=True, skip_group_check=True)

            # --- V transform ops (DVE) -------------------------------------
            def _vemit(j, o, e0, e1, o0, o1):
                if j == 0:
                    nc.vector.tensor_sub(o, e0, e1)
                elif j == 1:
                    nc.vector.tensor_add(o, o0, e1)
                elif j == 2:
                    nc.vector.tensor_sub(o, e1, o0)
                else:
                    nc.vector.tensor_sub(o, o0, o1)

            def vop0(j, dst, half):
                # s0: split layout [xe-p1|xo-p1|xe-p2|xo-p2]; half 0 covers V
                # rows 0-17 (entries 0:288), half 1 rows 18-33 (288:544).
                src = XAB[0] if dst is V[(0, j)] else XBD[0]
                if half == 0:
                    xe = src[:, 0:P1].rearrange("p (r c) -> p r c", c=17)
                    xo = src[:, P1 : 2 * P1].rearrange("p (r c) -> p r c", c=17)
                    o = dst[:, 0:288].rearrange("p (r c) -> p r c", c=16)
                else:
                    b = 2 * P1
                    xe = src[:, b : b + P2].rearrange("p (r c) -> p r c", c=17)
                    xo = src[:, b + P2 : b + 2 * P2].rearrange("p (r c) -> p r c", c=17)
                    o = dst[:, 288:544].rearrange("p (r c) -> p r c", c=16)
                _vemit(j, o, xe[:, :, 0:16], xe[:, :, 1:17],
                       xo[:, :, 0:16], xo[:, :, 1:17])

            def vop1(j, dst):
                # s1: plain layout [xe|xo] in cols 0:1156, one whole op
                src = XAB[1] if dst is V[(1, j)] else XBD[1]
                xe = src[:, 0:XSZ].rearrange("p (r c) -> p r c", c=17)
                xo = src[:, XSZ : 2 * XSZ].rearrange("p (r c) -> p r c", c=17)
                o = dst[:].rearrange("p (r c) -> p r c", c=16)
                _vemit(j, o, xe[:, :, 0:16], xe[:, :, 1:17],
                       xo[:, :, 0:16], xo[:, :, 1:17])

            for j in (1, 2, 3):
                vop0(j, V[(0, j)], 0)
                vop0(j, D[(0, j)], 0)
            for j in (1, 2, 3):
                vop0(j, V[(0, j)], 1)
                vop0(j, D[(0, j)], 1)
            for j in range(4):
                vop1(j, V[(1, j)])
            vop1(0, D[(1, 0)])

            # --- matmul emission -------------------------------------------
            def lhsT(s, j, b, c):
                if s == 0 and j == 0:
                    vtile = H1[:, 0:VSZ]
                    dtile = H2
                else:
                    vtile = V[(s, j)][:]
                    dtile = D[(s, j)]
                if c < 3:
                    stt = 128 * b + 16 * c
                    return vtile[:, stt : stt + 128]
                if c == 3:
                    return dtile[:, 128 * b : 128 * b + 128]
                return dtile[0:64, 128 * b + 32 : 128 * b + 160]

            def rhs(s, j, c):
                if s == 0 and j == 0:
                    if c < 3:
                        return H1[:, VSZ + c * 192 : VSZ + (c + 1) * 192]
                    if c == 3:
                        return H2[:, VSZ : VSZ + 192]
                    return H2[0:64, VSZ + 192 : VSZ + 384]
                col = (j * 5 + c) * 192
                if c == 4:
                    return Wt[s][0:64, col : col + 192]
                return Wt[s][:, col : col + 192]

            def mm(s, j, b, c):
                p, hh = b // 2, b % 2
                dst = m[(s, p, j)][:, 192 * hh : 192 * (hh + 1)]
                nc.tensor.matmul(dst, lhsT(s, j, b, c), rhs(s, j, c),
                                 start=(c == 0), stop=(c == 4),
                                 skip_group_check=True)

            # --- y transform + output --------------------------------------
            def t01_op(s, p):
                # HW: TensorTensor may read only ONE input from PSUM. Act
                # stages m1 to SBUF; DVE adds m0 (PSUM) + m1c (SBUF).
                nc.scalar.copy(MC[(s, p)][:, 0:384], m[(s, p, 1)][:])
                nc.vector.tensor_add(TT[(s, p)][:, 0:384],
                                     m[(s, p, 0)][:], MC[(s, p)][:, 0:384])

            def t2_op(s, p):
                # Act stages m2; Pool computes t2 = m1 - m2 in SBUF (keeps
                # DVE free for mid-phase work)
                nc.scalar.copy(MC[(s, p)][:, 384:768], m[(s, p, 2)][:])
                nc.gpsimd.tensor_sub(TT[(s, p)][:, 384:768],
                                     MC[(s, p)][:, 0:384], MC[(s, p)][:, 384:768])

            def y0_op(s, p):
                # s0: Pool add from SBUF (t01 + staged m2); keeps DVE free
                nc.gpsimd.tensor_add(ST[(s, p)][:, 0:384],
                                     TT[(s, p)][:, 0:384], MC[(s, p)][:, 384:768])
                nc.sync.dma_start(out_d[s, p, :, 0:384], ST[(s, p)][:, 0:384])

            def y1_op(s, p):
                # s0: Act stages m3, Pool computes y1 = t2 - m3 in SBUF
                nc.scalar.copy(MC[(s, p)][:, 768:1152], m[(s, p, 3)][:])
                nc.gpsimd.tensor_sub(ST[(s, p)][:, 384:768],
                                     TT[(s, p)][:, 384:768], MC[(s, p)][:, 768:1152])
                nc.sync.dma_start(out_d[s, p, :, 384:768], ST[(s, p)][:, 384:768])

            # Per (s, j): blocks b0/b2 live in different PSUM banks, so their
            # accumulation groups may interleave chunk-by-chunk; b1/b3 (same
            # banks as b0/b2) must start only after b0/b2 groups stop.
            # j0(s0): blocks b0/b2 (different banks) chunk-interleaved to
            # stretch the c0-c2 phase over the H2 arrival; j1+(s0): strict
            # block order b0,b1,b2,b3 matching the split-piece V arrival
            # (b0/b1 read only V entries 0:288 = half 0).
            def jphase(s, j, order=(0, 1, 2, 3)):
                if (s == 0 and j == 0) or (s == 1 and j == 0 and
                                           CFG.get("s1j0_il")):
                    for c in range(5):
                        mm(s, 0, 0, c)
                        mm(s, 0, 2, c)
                    for c in range(5):
                        mm(s, 0, 1, c)
                        mm(s, 0, 3, c)
                    return
                if s == 0 and j == 0:
                    for c in range(5):
                        mm(0, 0, 0, c)
                        mm(0, 0, 2, c)
                    for c in range(5):
                        mm(0, 0, 1, c)
                        mm(0, 0, 3, c)
                    return
                for b in order:
                    for c in range(5):
                        mm(s, j, b, c)

            def vd1(k):
                if CFG["vd1_pos"] == "mid":
                    vop1(k, D[(1, k)])

            jphase(0, 0)
            if CFG["s0_struct"] == "pair":
                for j in (1, 2, 3):
                    jphase(0, j, order=(0, 1))
                    if j == 1:
                        t01_op(0, 0)
                    if j == 2:
                        t2_op(0, 0)
                        vd1(1)
                        y0_op(0, 0)
                y1_op(0, 0)
                vd1(2)
                for j in (1, 2, 3):
                    jphase(0, j, order=(2, 3))
                    if j == 1:
                        t01_op(0, 1)
                    if j == 2:
                        t2_op(0, 1)
                        vd1(3)
                        y0_op(0, 1)
                y1_op(0, 1)
            else:
                for j in (1, 2, 3):
                    jphase(0, j)
                    if j == 1:
                        t01_op(0, 0)
                        t01_op(0, 1)
                    if j == 2:
                        t2_op(0, 0)
                        vd1(1)
                        y0_op(0, 0)
                        t2_op(0, 1)
                        vd1(2)
                        y0_op(0, 1)
                        vd1(3)
                y1_op(0, 0)
                y1_op(0, 1)
            if CFG["vd1_pos"] == "late":
                for k in (1, 2, 3):
                    vop1(k, D[(1, k)])

            jphase(1, 0)
            jphase(1, 1, order=CFG.get("s1j1_order", (0, 1, 2, 3)))
            for p in range(NP):
                t01_op(1, p)
            # j2(s1): pair p1 first so its y0/t2 DVE ops start 0.8us earlier
            jphase(1, 2, order=CFG.get("s1j2_order", (2, 3, 0, 1)))
            # Emit Act/Pool tail ops FIRST: cross-engine waits snapshot the
            # per-engine emission counters, so anything emitted after a DVE op
            # conservatively waits for it.
            nc.scalar.copy(MC[(1, 1)][:, 384:768], m[(1, 1, 2)][:])   # Act
            t2_op(1, 0)                                               # Act+Pool
            nc.gpsimd.tensor_add(ST[(1, 1)][:, 0:384],                # Pool y0(1,1)
                                 TT[(1, 1)][:, 0:384], MC[(1, 1)][:, 384:768])
            nc.vector.tensor_sub(TT[(1, 1)][:, 384:768],              # DVE t2(1,1)
                                 MC[(1, 1)][:, 0:384], m[(1, 1, 2)][:])
            nc.vector.tensor_add(ST[(1, 0)][:, 0:384],                # DVE y0(1,0)
                                 TT[(1, 0)][:, 0:384], m[(1, 0, 2)][:])
            nc.sync.dma_start(out_d[1, 0, :, 0:384], ST[(1, 0)][:, 0:384])
            # final j-phase: pair p1 first (strict per-bank group order), so
            # y1(1,1) computes and its merged pair-DMA ships while p0's blocks
            # still stream; only one [128,384] y1 op remains after the last mm.
            for b in (2, 3, 0, 1):
                for c in range(5):
                    mm(1, 3, b, c)
                if b == 3:
                    nc.vector.tensor_sub(ST[(1, 1)][:, 384:768],      # DVE y1(1,1)
                                         TT[(1, 1)][:, 384:768], m[(1, 1, 3)][:])
                    nc.sync.dma_start(out_d[1, 1, :, 0:768], ST[(1, 1)][:])
            nc.vector.tensor_sub(ST[(1, 0)][:, 384:768],              # DVE y1(1,0)
                                 TT[(1, 0)][:, 384:768], m[(1, 0, 3)][:])
            nc.sync.dma_start(out_d[1, 0, :, 384:768], ST[(1, 0)][:, 384:768])

    nc.compile()
    return nc


def get_module():
    if "nc" not in _cache:
        _cache["nc"] = _build_module()
    return _cache["nc"]


def _route(x, gate_w, gate_b):
    """Replicates the reference router in numpy fp32. Returns combine [B,E]."""
    pooled = x.mean(axis=(2, 3), dtype=np.float32)
    logits = pooled @ gate_w + gate_b
    z = logits - logits.max(axis=-1, keepdims=True)
    ez = np.exp(z)
    w = ez / ez.sum(axis=-1, keepdims=True)
    topi = np.argsort(-w, axis=-1, kind="stable")[:, :TOPK]
    topw = np.take_along_axis(w, topi, axis=-1)
    topw = topw / (topw.sum(-1, keepdims=True) + 1e-10)
    combine = np.zeros((B, E), np.float32)
    np.put_along_axis(combine, topi, topw, axis=-1)
    return combine


def make_in_maps(x, gate_w, gate_b, expert_w, expert_b):
    import ml_dtypes

    bf16 = ml_dtypes.bfloat16
    x = np.ascontiguousarray(np.asarray(x, np.float32))
    gate_w = np.asarray(gate_w, np.float32)
    gate_b = np.asarray(gate_b, np.float32)
    expert_w = np.asarray(expert_w, np.float32)
    expert_b = np.asarray(expert_b, np.float32)

    combine = _route(x, gate_w, gate_b)                       # [B,E]
    Wc = np.einsum("be,eoikl->boikl", combine, expert_w)      # [B,C,C,3,3]
    bc = combine @ expert_b                                   # [B,C]

    # U_j[b, dy, ich, och]: G-transformed weights
    A = Wc.transpose(0, 3, 2, 1, 4)                           # [b, dy, i, o, dx]
    U = np.stack([
        A[..., 0],
        (A[..., 0] + A[..., 1] + A[..., 2]) * 0.5,
        (A[..., 0] - A[..., 1] + A[..., 2]) * 0.5,
        A[..., 2],
    ])                                                        # [j, b, dy, i, o]
    U = U.astype(bf16)

    u = np.zeros((B, 128, 4 * UW), bf16)
    for j in range(4):
        for c in range(3):
            u[:, :, (j * 5 + c) * 192 : (j * 5 + c + 1) * 192] = U[j, :, c, 0:128]
        col = (j * 5 + 3) * 192
        u[:, 0:64, col : col + 192] = U[j, :, 0, 128:192]
        u[:, 64:128, col : col + 192] = U[j, :, 1, 128:192]
        col = (j * 5 + 4) * 192
        u[:, 0:64, col : col + 192] = U[j, :, 2, 128:192]

    # de-interleaved padded image
    xp34 = np.zeros((B, C, 34, 34), np.float32)
    xp34[:, :, 1:33, 1:33] = x
    xe = xp34[:, :, :, 0::2].astype(bf16)                     # [B,C,34,17]
    xo = xp34[:, :, :, 1::2].astype(bf16)
    xef = xe.reshape(B, C, XSZ)
    xof = xo.reshape(B, C, XSZ)
    # shifted-by-one-V-row copies (rows 1..34, row 34 = zeros)
    zrow = np.zeros((B, C, 1, 17), bf16)
    xes = np.concatenate([xe[:, :, 1:, :], zrow], axis=2).reshape(B, C, XSZ)
    xos = np.concatenate([xo[:, :, 1:, :], zrow], axis=2).reshape(B, C, XSZ)

    # plain layout [xe | xo] (used by odd per-core samples)
    xab = np.zeros((B, 128, XW), bf16)
    xab[:, :, 0:XSZ] = xef[:, 0:128]
    xab[:, :, XSZ:] = xof[:, 0:128]
    xbd = np.zeros((B, 128, XW), bf16)
    xbd[:, 0:64, 0:XSZ] = xef[:, 128:192]
    xbd[:, 64:128, 0:XSZ] = xes[:, 128:192]
    xbd[:, 0:64, XSZ:] = xof[:, 128:192]
    xbd[:, 64:128, XSZ:] = xos[:, 128:192]

    def to_split(t):
        # [*, 128, 1156] plain [xe|xo] -> split [xe-p1|xo-p1|xe-p2|xo-p2]
        out = np.empty_like(t)
        out[..., 0:P1] = t[..., 0:P1]
        out[..., P1 : 2 * P1] = t[..., XSZ : XSZ + P1]
        out[..., 2 * P1 : 2 * P1 + P2] = t[..., P1:XSZ]
        out[..., 2 * P1 + P2 :] = t[..., XSZ + P1 :]
        return out

    # host-side V_0 for each core's sample 0 (head tiles)
    xe32 = xe.astype(np.float32)
    v0 = (xe32[:, :, :, 0:16] - xe32[:, :, :, 1:17]).astype(bf16)  # [B,C,34,16]
    v0f = v0.reshape(B, C, VSZ)
    v0s = np.concatenate([v0f[:, :, 16:],
                          np.zeros((B, C, 16), bf16)], axis=2)

    in_maps = []
    for cidx in range(NCORES):
        b0 = S * cidx
        h1 = np.zeros((1, 128, VSZ + 3 * 192), bf16)
        h1[0, :, 0:VSZ] = v0f[b0, 0:128]
        h1[0, :, VSZ:] = u[b0, :, 0 : 3 * 192]
        h2 = np.zeros((1, 128, VSZ + 2 * 192), bf16)
        h2[0, 0:64, 0:VSZ] = v0f[b0, 128:192]
        h2[0, 64:128, 0:VSZ] = v0s[b0, 128:192]
        h2[0, :, VSZ:] = u[b0, :, 3 * 192 : 5 * 192]
        xab_c = xab[b0 : b0 + S].copy()
        xbd_c = xbd[b0 : b0 + S].copy()
        xab_c[0] = to_split(xab_c[0])
        xbd_c[0] = to_split(xbd_c[0])
        in_maps.append(
            {
                "xab": xab_c,
                "xbd": xbd_c,
                "u": np.ascontiguousarray(u[b0 : b0 + S]),
                "h1": h1,
                "h2": h2,
            }
        )
    return in_maps, bc


def postprocess(dev_out, bc_rows):
    """[S, NP, 128, 768] bf16 -> [S,C,H,W] f32."""
    dev = np.asarray(dev_out, np.float32)
    # [p, pi, y, h, och] -> pos = 128*(2p+h)+pi
    a = dev.reshape(S, NP, 128, 2, 2, 192)
    a = a.transpose(0, 1, 4, 2, 5, 3)        # [s, p, h, pi, och, y]
    a = a.reshape(S, 32, 16, 192, 2)         # [s, r, t, och, y]
    out = a.transpose(0, 3, 1, 2, 4).reshape(S, C, H, W)
    return out + bc_rows[:, :, None, None]


def kernel(x, gate_w, gate_b, expert_w, expert_b):
    from concourse.bass_utils import run_bass_kernel_spmd

    nc = get_module()
    in_maps, bc = make_in_maps(x, gate_w, gate_b, expert_w, expert_b)
    res = run_bass_kernel_spmd(nc, in_maps, core_ids=list(range(NCORES)))
    out = np.empty((B, C, H, W), np.float32)
    for c in range(NCORES):
        b0 = S * c
        out[b0 : b0 + S] = postprocess(res.results[c]["out"], bc[b0 : b0 + S])
    return out
